# revision 43
# baseline (speedup 1.0000x reference)
"""Multi-head causal attention (B=2, S=2048, D=1024, H=16) on 8 TRN2 NeuronCores.

Sharding: (batch, head-group). Core c handles batch c//4 and heads
[4*(c%4) .. 4*(c%4)+3]:
  - Wq/Wk/Wv column-sliced [1024, 256] per core -> per-core q,k,v (4 heads)
  - causal attention for the 4 local heads (scoresT layout)
  - Wo row-sliced [256, 1024] -> bf16 partial output [2048, 1024] per core
  - host sums the 4 partials per batch (+bo) = exact all-reduce

Versus head-only sharding this halves the input DMA (one batch of xT) and
halves the partial-output DMA (written bf16), and leaves a single batch
stream that pipelines cleanly.

Schedule: the kernel is a sequence of attention i-chunks (512 rows). The
q/k/v projection for chunk jc+1 and the output projection for chunk jc-1 are
emitted as background tensor groups interleaved between attention score
pairs of chunk jc, so the tensor engine never idles while the scalar engine
works through the exp stream (and vice versa).

Scores are computed transposed (scoresT[j, i] = k_j . q_i); exp runs on
pairs of j-tiles (one activation over a 2-bank PSUM region) to halve
activation instruction overhead; the attn@V matmul consumes at directly as
the moving operand with V stationary, and a ones-column appended to V
yields the softmax denominator for free (row 64 of the ctx PSUM). The
per-i reciprocal is broadcast to head rows via a tiny [33,128] matmul
(exact bf16 hi/lo split). Softmax skips max-subtraction: scores/8 ~
N(0,0.4), exp cannot overflow. All matmuls run bf16 with f32 PSUM.
"""

import numpy as np

B, S, D = 2, 2048, 1024
H, HD = 16, 64
NCORES = 8
HLOC = 4                 # heads per core
DLOC = HLOC * HD         # local qkv width = 256
SB = S                   # rows per core (one batch)
IC = SB // 512           # 4 i-chunks of 512
JT = SB // 128           # 16 j-tiles of 128
KT = D // 128            # 8 contraction tiles for projections

_CACHE = {}


def _install_ntff_hook():
    import sys, types
    if "antenv.axon_hooks" in sys.modules:
        return
    mod = types.ModuleType("antenv.axon_hooks")
    mod._hook = None
    mod.set_axon_ntff_profile_hook = lambda h: setattr(mod, "_hook", h)
    mod.get_axon_ntff_profile_hook = lambda: mod._hook
    sys.modules["antenv.axon_hooks"] = mod
    import antenv
    antenv.axon_hooks = mod
    try:
        from trn_agent_boot.trn_boot import _ntff_profile_via_ctypes
        mod.set_axon_ntff_profile_hook(
            _ntff_profile_via_ctypes("/opt/axon/libaxon_pjrt.so"))
    except Exception:
        pass


def _build():
    import concourse.bass as bass
    import concourse.tile as tile
    from concourse import bacc, mybir

    f32 = mybir.dt.float32
    bf16 = mybir.dt.bfloat16
    f32r = mybir.dt.float32r
    EXP = mybir.ActivationFunctionType.Exp

    nc = bacc.Bacc("TRN2", target_bir_lowering=False, debug=False,
                   num_devices=NCORES)
    # xt: jc-major [128, IC*4096]; col = jc*4096 + kt*512 + r  (r in-chunk)
    xt_d = nc.dram_tensor("xt", [128, KT * SB], bf16, kind="ExternalInput").ap()
    # wq/wk/wv: kt-major [128, KT*256]
    wq_d = nc.dram_tensor("wq", [128, KT * DLOC], bf16, kind="ExternalInput").ap()
    wk_d = nc.dram_tensor("wk", [128, KT * DLOC], bf16, kind="ExternalInput").ap()
    wv_d = nc.dram_tensor("wv", [128, KT * DLOC], bf16, kind="ExternalInput").ap()
    # wo: row-blocked [128, 2*1024]
    wo_d = nc.dram_tensor("wo", [128, 2 * D], bf16, kind="ExternalInput").ap()
    out_d = nc.dram_tensor("out", [SB, D], bf16, kind="ExternalOutput").ap()

    with tile.TileContext(nc) as tc:
        with tc.tile_pool(name="const", bufs=1) as cpool, \
             tc.tile_pool(name="w", bufs=1) as wpool, \
             tc.tile_pool(name="xt", bufs=1) as xtpool, \
             tc.tile_pool(name="qk", bufs=1) as qkpool, \
             tc.tile_pool(name="ve", bufs=1) as vepool, \
             tc.tile_pool(name="at", bufs=6) as atpool, \
             tc.tile_pool(name="cx", bufs=2) as cxpool, \
             tc.tile_pool(name="dn", bufs=3) as dnpool, \
             tc.tile_pool(name="sm", bufs=3) as smpool, \
             tc.tile_pool(name="ot", bufs=6) as otpool, \
             tc.tile_pool(name="ps", bufs=2, space="PSUM") as ps_s, \
             tc.tile_pool(name="pc", bufs=2, space="PSUM") as ps_c, \
             tc.tile_pool(name="pm", bufs=2, space="PSUM") as ps_m:

            # ---- constants ----
            # E: bcast matrix, row 0 -> out rows 0:64, row 32 -> rows 64:128
            e_f = cpool.tile([64, 128], f32, tag="e_f")
            nc.gpsimd.memset(e_f[:], 0.0)
            nc.gpsimd.affine_select(
                out=e_f[0:32, :], in_=e_f[0:32, :],
                compare_op=mybir.AluOpType.is_ge,
                fill=1.0, base=-64, pattern=[[1, 128]], channel_multiplier=64)
            nc.gpsimd.affine_select(
                out=e_f[32:64, :], in_=e_f[32:64, :],
                compare_op=mybir.AluOpType.is_ge,
                fill=1.0, base=63, pattern=[[-1, 128]], channel_multiplier=64)
            emat = cpool.tile([33, 128], f32r, tag="emat")
            nc.vector.tensor_copy(emat[:], e_f[0:33, :])

            # ---- weights + xt DMA (order chosen so chunk-0 compute can
            # start after ~2MB) ----
            wq_sb = wpool.tile([128, KT * DLOC], bf16, tag="wq")
            wk_sb = wpool.tile([128, KT * DLOC], bf16, tag="wk")
            wv_sb = wpool.tile([128, KT * DLOC], bf16, tag="wv")
            wo_sb = wpool.tile([128, 2 * D], bf16, tag="wo")
            xt_sb = xtpool.tile([128, KT * SB], bf16, tag="xt")
            nc.sync.dma_start(wq_sb[:], wq_d[:])
            nc.sync.dma_start(xt_sb[:, 0:2048], xt_d[:, 0:2048])
            nc.sync.dma_start(xt_sb[:, 2048:4096], xt_d[:, 2048:4096])
            nc.sync.dma_start(wk_sb[:], wk_d[:])
            nc.sync.dma_start(wv_sb[:], wv_d[:])
            nc.sync.dma_start(xt_sb[:, 4096:8192], xt_d[:, 4096:8192])
            nc.sync.dma_start(wo_sb[:], wo_d[:])
            for jc in range(2, IC):
                nc.sync.dma_start(xt_sb[:, jc * 4096:(jc + 1) * 4096],
                                  xt_d[:, jc * 4096:(jc + 1) * 4096])

            def xts(kt, a, b):
                jc, r = divmod(a, 512)
                off = jc * 4096 + kt * 512 + r
                return xt_sb[:, off:off + (b - a)]

            # persistent q/k (scoresT layout) and v-ext tiles
            qt = [qkpool.tile([128, SB], bf16, tag=f"q{g}", name=f"qt{g}")
                  for g in range(2)]
            kt_t = [qkpool.tile([128, SB], bf16, tag=f"k{g}", name=f"ktt{g}")
                    for g in range(2)]
            # ve: [128, 4*1040]; col h*1040 + jt*65 + d, d=64 is the ones col
            ve = vepool.tile([128, HLOC * 65 * JT], bf16, tag="ve")
            vev = ve[:].rearrange("p (h j c) -> p h j c", h=HLOC, c=65)
            for h in range(HLOC):
                nc.gpsimd.memset(vev[:, h, :, 64], 1.0)

            # ---- background tensor groups (emitted between score pairs) ---
            def qk_group(jc, w_sb, dest, g, scalar_cast=False):
                def emit():
                    c0 = jc * 512
                    P = ps_m.tile([128, 512], f32, tag="m", name="Pqk")
                    for kt in range(KT):
                        nc.tensor.matmul(
                            P[:],
                            w_sb[:, g * 1024 + kt * 128:
                                 g * 1024 + (kt + 1) * 128],
                            xts(kt, c0, c0 + 512),
                            start=(kt == 0), stop=(kt == KT - 1))
                    if scalar_cast:
                        nc.scalar.copy(dest[g][:, c0:c0 + 512], P[:])
                    else:
                        nc.vector.tensor_copy(dest[g][:, c0:c0 + 512], P[:])
                return emit

            def v_group(jc, jp):
                def emit():
                    Pv = ps_m.tile([128, 512], f32, tag="m", name="Pv")
                    for sub in range(2):
                        jt = jc * 4 + jp * 2 + sub
                        for kt in range(KT):
                            nc.tensor.matmul(
                                Pv[:, sub * 256:(sub + 1) * 256],
                                xts(kt, jt * 128, (jt + 1) * 128),
                                wv_sb[:, kt * DLOC:(kt + 1) * DLOC],
                                start=(kt == 0), stop=(kt == KT - 1),
                                skip_group_check=True)
                    for sub in range(2):
                        jt = jc * 4 + jp * 2 + sub
                        src = Pv[:, sub * 256:(sub + 1) * 256].rearrange(
                            "p (h d) -> p h d", h=HLOC)
                        nc.vector.tensor_copy(vev[:, :, jt, 0:64], src)
                return emit

            def proj_groups(jc, scalar_casts=0):
                gs = []
                k = 0
                for w_sb, dest in ((wq_sb, qt), (wk_sb, kt_t)):
                    for g in range(2):
                        gs.append(qk_group(jc, w_sb, dest, g,
                                           scalar_cast=(k < scalar_casts)))
                        k += 1
                for jp in range(2):
                    gs.append(v_group(jc, jp))
                return gs

            def outproj_units(ic, cxs, alt=False):
                us = []
                c0 = ic * 512
                for isl in range(4):
                    ot = otpool.tile([128, D], bf16, tag="ot", name="ot")

                    def unit(isl=isl, ot=ot):
                        for nk in range(2):
                            Po = ps_m.tile([128, 512], f32, tag="m", name="Po")
                            nc.tensor.matmul(
                                Po[:], cxs[0][:, isl * 128:(isl + 1) * 128],
                                wo_sb[:, nk * 512:(nk + 1) * 512],
                                start=True, stop=False, skip_group_check=True)
                            nc.tensor.matmul(
                                Po[:], cxs[1][:, isl * 128:(isl + 1) * 128],
                                wo_sb[:, D + nk * 512:D + (nk + 1) * 512],
                                start=False, stop=True, skip_group_check=True)
                            if alt and nk == 0:
                                nc.scalar.copy(
                                    ot[:, nk * 512:(nk + 1) * 512], Po[:])
                            else:
                                nc.vector.tensor_copy(
                                    ot[:, nk * 512:(nk + 1) * 512], Po[:])
                        nc.sync.dma_start(
                            out_d[c0 + isl * 128:c0 + (isl + 1) * 128, :],
                            ot[:])
                    us.append(unit)
                return us

            def outproj_subunits(ic, cxs, alt=False):
                # finer-grained (per-nk) units for interleaving into the
                # scalar-bound chunks
                us = []
                c0 = ic * 512
                for isl in range(4):
                    ot = otpool.tile([128, D], bf16, tag="ot", name="ot")
                    for nk in range(2):
                        def su(isl=isl, nk=nk, ot=ot):
                            Po = ps_m.tile([128, 512], f32, tag="m",
                                           name="Po")
                            nc.tensor.matmul(
                                Po[:], cxs[0][:, isl * 128:(isl + 1) * 128],
                                wo_sb[:, nk * 512:(nk + 1) * 512],
                                start=True, stop=False, skip_group_check=True)
                            nc.tensor.matmul(
                                Po[:], cxs[1][:, isl * 128:(isl + 1) * 128],
                                wo_sb[:, D + nk * 512:D + (nk + 1) * 512],
                                start=False, stop=True, skip_group_check=True)
                            if alt and nk == 0:
                                nc.scalar.copy(
                                    ot[:, nk * 512:(nk + 1) * 512], Po[:])
                            else:
                                nc.vector.tensor_copy(
                                    ot[:, nk * 512:(nk + 1) * 512], Po[:])
                            if nk == 1:
                                nc.sync.dma_start(
                                    out_d[c0 + isl * 128:
                                          c0 + (isl + 1) * 128, :], ot[:])
                        us.append(su)
                return us

            # ---- attention i-chunk with background interleave ----
            def attention(jc, bg, last=False):
                c0 = jc * 512
                npair = 2 * jc + 2
                total_pairs = 4 * npair
                stride = max(1, -(-total_pairs // max(1, len(bg))))
                state = {"pcount": 0}
                cxs = []
                chains = []
                for _ in range(1):
                    if bg:
                        bg.pop(0)()
                for hp in range(2):
                    if hp == 1:
                        for _ in range(2):
                            if bg:
                                bg.pop(0)()
                    den = dnpool.tile([33, 512], f32, tag="den", name="den")
                    nc.gpsimd.memset(den[:], 1.0)
                    ctxT = cxpool.tile([128, 512], f32, tag=f"ct{hp}",
                                       name=f"ct{hp}")
                    # the two heads' pair streams are interleaved so the
                    # exp pipeline never drains at a head boundary
                    Pcs = [ps_c.tile([65, 512], f32, tag="ctx",
                                     name=f"Pc{h}") for h in range(2)]
                    pends = [[], []]

                    # one-pair-deep per stream (= two global pairs): the exp
                    # latency (~1.1us) exceeds one pair's tensor time
                    def flush(n, h):
                        Pc, pend = Pcs[h], pends[h]
                        while len(pend) > n:
                            for (at_, jt_, e0_, sub_) in pend.pop(0):
                                nc.tensor.matmul(
                                    Pc[:, e0_:512],
                                    vev[:, hp * 2 + h, jt_, :],
                                    at_[:, sub_ * 512 + e0_:
                                        (sub_ + 1) * 512],
                                    start=(jt_ == 0),
                                    stop=(jt_ == 4 * jc + 3),
                                    skip_group_check=True)

                    for p in range(npair):
                        for h in range(2):
                            Ps = ps_s.tile([128, 1024], f32, tag="s",
                                           name="Ps")
                            info = []
                            for sub in range(2):
                                jt = 2 * p + sub
                                kb = jt - 4 * jc
                                e0 = 0 if kb < 0 else 128 * kb
                                nc.tensor.matmul(
                                    Ps[:, sub * 512 + e0:(sub + 1) * 512],
                                    kt_t[hp][h * 64:(h + 1) * 64,
                                             jt * 128:(jt + 1) * 128],
                                    qt[hp][h * 64:(h + 1) * 64,
                                           c0 + e0:c0 + 512],
                                    start=True, stop=True,
                                    skip_group_check=True)
                                info.append((jt, kb, e0))
                            at = atpool.tile([128, 1024], bf16, tag="at",
                                             name="at")
                            e0L = info[0][2]
                            nc.scalar.activation(
                                at[:, e0L:1024], Ps[:, e0L:1024], EXP,
                                scale=0.125)
                            for sub in range(2):
                                jt, kb, e0 = info[sub]
                                if kb >= 0:
                                    nc.gpsimd.affine_select(
                                        out=at[:, sub * 512 + e0:
                                               sub * 512 + e0 + 128],
                                        in_=at[:, sub * 512 + e0:
                                               sub * 512 + e0 + 128],
                                        compare_op=mybir.AluOpType.is_ge,
                                        fill=0.0, base=0, pattern=[[1, 128]],
                                        channel_multiplier=-1)
                            flush(1, h)
                            pends[h].append(
                                [(at, info[sub][0], info[sub][2], sub)
                                 for sub in range(2)])
                            state["pcount"] += 1
                            if bg and state["pcount"] % stride == 0:
                                bg.pop(0)()
                    for h in range(2):
                        flush(0, h)
                        # in the last chunk the scalar engine is winding
                        # down; give it the PSUM drains to shorten the tail
                        if last and hp == 1:
                            # scalar is drained here; parallelize the two
                            # copies across scalar and vector
                            nc.scalar.copy(den[h * 32:h * 32 + 1, :],
                                           Pcs[h][64:65, :])
                            nc.vector.tensor_copy(
                                ctxT[h * 64:(h + 1) * 64, :], Pcs[h][0:64, :])
                        else:
                            nc.vector.tensor_copy(den[h * 32:h * 32 + 1, :],
                                                  Pcs[h][64:65, :])
                            nc.vector.tensor_copy(
                                ctxT[h * 64:(h + 1) * 64, :], Pcs[h][0:64, :])
                    rr = smpool.tile([33, 512], f32, tag="rr", name="rr")
                    nc.vector.reciprocal_approx_fast(rr[:], den[:])
                    rrr = smpool.tile([33, 512], f32r, tag="rrr", name="rrr")
                    nc.vector.tensor_copy(rrr[:], rr[:])
                    chains.append((ctxT, rrr))
                    if last:
                        # emit the normalize chain immediately: the tensor
                        # engine is scalar-paced here and the tail shortens
                        ctxT, rrr = chains[hp]
                        Pb = ps_m.tile([128, 512], f32, tag="m", name="Pb")
                        nc.tensor.matmul(Pb[:], emat[:], rrr[:], start=True,
                                         stop=True, skip_group_check=True)
                        cx = cxpool.tile([128, 512], bf16, tag=f"cx{hp}",
                                         name=f"cx{hp}")
                        nc.vector.tensor_mul(cx[:], ctxT[:], Pb[:])
                        cxs.append(cx)
                # leftover background groups
                while bg:
                    bg.pop(0)()
                if last:
                    return cxs
                # normalize: cx = ctxT * (1/den) broadcast via E matmul
                for hp in range(2):
                    ctxT, rrr = chains[hp]
                    Pb = ps_m.tile([128, 512], f32, tag="m", name="Pb")
                    nc.tensor.matmul(Pb[:], emat[:], rrr[:], start=True,
                                     stop=True, skip_group_check=True)
                    cx = cxpool.tile([128, 512], bf16, tag=f"cx{hp}",
                                     name=f"cx{hp}")
                    nc.vector.tensor_mul(cx[:], ctxT[:], Pb[:])
                    cxs.append(cx)
                return cxs

            # ---- main schedule ----
            # chunk-0 q-projection split in kt halves so the PE starts as
            # soon as the first half of the chunk-0 xt DMA lands
            Pq0 = []
            for g in range(2):
                P = ps_m.tile([128, 512], f32, tag="m", name="Pqk0")
                Pq0.append(P)
                for kt in range(KT // 2):
                    nc.tensor.matmul(
                        P[:],
                        wq_sb[:, g * 1024 + kt * 128:g * 1024 + (kt + 1) * 128],
                        xts(kt, 0, 512), start=(kt == 0), stop=False,
                        skip_group_check=True)
            for g in range(2):
                for kt in range(KT // 2, KT):
                    nc.tensor.matmul(
                        Pq0[g][:],
                        wq_sb[:, g * 1024 + kt * 128:g * 1024 + (kt + 1) * 128],
                        xts(kt, 0, 512), start=False, stop=(kt == KT - 1),
                        skip_group_check=True)
                if g == 0:
                    nc.scalar.copy(qt[g][:, 0:512], Pq0[g][:])
                else:
                    nc.vector.tensor_copy(qt[g][:, 0:512], Pq0[g][:])
            for g in proj_groups(0, scalar_casts=3)[2:]:
                g()
            cxs_hist = {}
            for jc in range(IC):
                bg = []
                if jc == 0:
                    bg += proj_groups(1, scalar_casts=2)
                elif jc == 1:
                    bg += proj_groups(2, scalar_casts=2)
                    bg += outproj_subunits(0, cxs_hist[0], alt=True)
                elif jc == 2:
                    bg += proj_groups(3, scalar_casts=2)
                else:
                    bg += outproj_subunits(1, cxs_hist[1])
                    bg += outproj_subunits(2, cxs_hist[2])
                cxs_hist[jc] = attention(jc, bg, last=(jc == IC - 1))
            for u in outproj_units(IC - 1, cxs_hist[IC - 1], alt=True):
                u()

    nc.compile()
    return nc


def _get_nc():
    if "nc" not in _CACHE:
        _install_ntff_hook()
        _CACHE["nc"] = _build()
    return _CACHE["nc"]


def _run(inputs, trace=False):
    from concourse.bass_utils import run_bass_kernel_spmd
    import ml_dtypes

    nc = _get_nc()
    x = np.asarray(inputs["x"], dtype=np.float32)
    Wq = np.asarray(inputs["Wq"], dtype=np.float32)
    Wk = np.asarray(inputs["Wk"], dtype=np.float32)
    Wv = np.asarray(inputs["Wv"], dtype=np.float32)
    Wo = np.asarray(inputs["Wo"], dtype=np.float32)
    bo = np.asarray(inputs["bo"], dtype=np.float32)
    bf = ml_dtypes.bfloat16

    def kt_major(a):
        # [1024, 256] -> [128, KT*256] kt-major per partition
        return np.ascontiguousarray(
            a.reshape(KT, 128, DLOC).transpose(1, 0, 2)
            .reshape(128, KT * DLOC)).astype(bf)

    def g_major(a):
        # [1024, 256] -> [128, 2*1024]; col = g*1024 + kt*128 + c
        return np.ascontiguousarray(
            a.reshape(KT, 128, 2, 128).transpose(1, 2, 0, 3)
            .reshape(128, KT * DLOC)).astype(bf)

    xts = []
    for b in range(B):
        # [1024, 2048] -> [128, jc*4096 + kt*512 + r]
        xtb = np.ascontiguousarray(x[b].T)
        xtb = xtb.reshape(KT, 128, IC, 512).transpose(1, 2, 0, 3)
        xts.append(np.ascontiguousarray(xtb.reshape(128, KT * SB)).astype(bf))

    in_maps = []
    for c in range(NCORES):
        b, hg = divmod(c, 4)
        sl = slice(hg * DLOC, (hg + 1) * DLOC)
        in_maps.append({
            "xt": xts[b],
            "wq": g_major(Wq[:, sl]),
            "wk": g_major(Wk[:, sl]),
            "wv": kt_major(Wv[:, sl]),
            "wo": np.ascontiguousarray(
                Wo[sl, :].reshape(2, 128, D).transpose(1, 0, 2)
                .reshape(128, 2 * D)).astype(bf),
        })
    res = run_bass_kernel_spmd(nc, in_maps, core_ids=list(range(NCORES)),
                               trace=trace)
    out = np.zeros((B, SB, D), dtype=np.float32)
    for c in range(NCORES):
        b = c // 4
        out[b] += res.results[c]["out"].astype(np.float32)
    out += bo[None, None, :]
    return out, res


def kernel(**inputs):
    out, _ = _run(inputs, trace=False)
    return out
